# revision 17
# baseline (speedup 1.0000x reference)
"""Trainium2 Bass kernel for nn_PlatonicConv (linear-attention GNN message passing).

Math (reference):
  q = rope(x@Wq + bq, phase);  k = rope(ones, phase);  v = x@Wv + bv
  phase[n, g, p] = pos[n, :] . freqs[g, 0, p, :]
  KV_b[g] = (1/AVG) * sum_{n in graph b} k[n,g,:] (x) v[n,g,:]
  out[n]  = concat_g( q'[n,g,:] @ KV_b[g] ) @ Wo + bo

Device formulation (per core, data-parallel over graphs; 8 graphs/core):
  - host precomputes trig: cos/sin of phase (feature-major "layout L" for the
    q rope) and k = rope(ones) (node-major, 1/sqrt2 folded into Wo scale).
    (ACT's Sin LUT is only valid on [-pi, pi]; phases exceed that.)
  - Per graph b:  M_b = stack_g(KV_b[g] @ Wo[g-rows]) : [384, 384]
    out[n] = q'[n] @ M_{b(n)}  (+ bo on host).
  - q'/M_b rows use "A-order" over rope pairs t = g*16+p:
      rows   0:128 = E_t (even dims), t=0..127
      rows 128:256 = O_t (odd  dims), t=0..127
      rows 256:320 = E_t, t=128..191   (separate 64-partition tiles)
      rows 320:384 = O_t, t=128..191
  - k/v tiles are node-major; KV^T computed with dense 4-group blocks on PE;
    diagonal 32x32 blocks copied into pre-zeroed block-diagonal "arena"
    stationaries for the col-tiled M_b matmuls.

Self-contained: hardcodes shapes; shards/pads on host inside kernel().
"""

import math
import os
from contextlib import ExitStack

import ml_dtypes
import numpy as np

import concourse.bacc as bacc_mod
import concourse.bass as bass
import concourse.mybir as mybir
import concourse.tile as tile
from concourse.bass_utils import run_bass_kernel_spmd


def _ensure_ntff_hook():
    """Register the axon NTFF profile hook if the image's antenv lacks it."""
    try:
        import antenv.axon_hooks  # noqa: F401

        return True
    except ImportError:
        pass
    try:
        import sys
        import types

        import antenv
        from trn_agent_boot.trn_boot import _ntff_profile_via_ctypes

        mod = types.ModuleType("antenv.axon_hooks")
        _hook = [None]
        mod.set_axon_ntff_profile_hook = lambda h: _hook.__setitem__(0, h)
        mod.get_axon_ntff_profile_hook = lambda: _hook[0]
        sys.modules["antenv.axon_hooks"] = mod
        antenv.axon_hooks = mod
        mod.set_axon_ntff_profile_hook(
            _ntff_profile_via_ctypes("/opt/axon/libaxon_pjrt.so")
        )
        return True
    except Exception:
        return False


FP32 = mybir.dt.float32
BF16 = mybir.dt.bfloat16
AF = mybir.ActivationFunctionType

N = 32768
C = 384
E = 384
G = 12
D = 32
P = 16
SD = 3
NUM_GRAPHS = 64
NCORES = 8
GPD = NUM_GRAPHS // NCORES  # graphs per device
AVG = float(N) / NUM_GRAPHS  # 512.0
NT = 192  # rope pairs = G*P
W = 512  # streaming window


def _a_order_cols():
    """perm such that A-order column r is original q-dim perm[r]."""
    perm = np.empty(E, dtype=np.int64)
    for r in range(E):
        if r < 128:
            t, odd = r, 0
        elif r < 256:
            t, odd = r - 128, 1
        elif r < 320:
            t, odd = 128 + (r - 256), 0
        else:
            t, odd = 128 + (r - 320), 1
        perm[r] = (t // 16) * 32 + 2 * (t % 16) + odd
    return perm


_APERM = _a_order_cols()

_CACHE = {}


def _build(slot: int, has_bias: bool):
    key = (slot, has_bias)
    if key in _CACHE:
        return _CACHE[key]

    NP = GPD * slot
    NTILE = NP // 128
    TPS = slot // 128
    NCH = NP // W
    assert NP % W == 0

    nc = bacc_mod.Bacc()

    CA = C + 1 if has_bias else C
    blocks = [128, 128, 128] + ([1] if has_bias else [])
    nk = len(blocks)
    rowoff = [0, 128, 256, 384]

    xt_d = nc.declare_dram_parameter("xt", [CA, NP], BF16, isOutput=False)
    cl_d = nc.declare_dram_parameter("cl", [NT, NP], BF16, isOutput=False)
    sl_d = nc.declare_dram_parameter("sl", [NT, NP], BF16, isOutput=False)
    kn_d = nc.declare_dram_parameter("kn", [NP, E], BF16, isOutput=False)
    wqa_d = nc.declare_dram_parameter("wqa", [CA, E], BF16, isOutput=False)
    wva_d = nc.declare_dram_parameter("wva", [CA, E], BF16, isOutput=False)
    wos_d = nc.declare_dram_parameter("wos", [E, C], BF16, isOutput=False)
    out_d = nc.declare_dram_parameter("outt", [3, 128, NP], FP32, isOutput=True)

    with ExitStack() as ctx:
        tc = ctx.enter_context(tile.TileContext(nc))

        consts = ctx.enter_context(tc.tile_pool(name="consts", bufs=1))
        xtp = ctx.enter_context(tc.tile_pool(name="xtp", bufs=3))
        qsb = ctx.enter_context(tc.tile_pool(name="qsb", bufs=2))
        big = ctx.enter_context(tc.tile_pool(name="big", bufs=1))
        aren = ctx.enter_context(tc.tile_pool(name="aren", bufs=1))
        mbp = ctx.enter_context(tc.tile_pool(name="mbp", bufs=2))
        outp = ctx.enter_context(tc.tile_pool(name="outp", bufs=4))
        psum = ctx.enter_context(tc.tile_pool(name="psum", bufs=1, space="PSUM"))

        def pbank(tag):
            """Allocate one full PSUM bank ([128, 512] f32)."""
            return psum.tile([128, W], FP32, tag=tag, name=tag)

        # ---- constants (weights) ----
        wq_t, wv_t = [], []
        for bi, rows in enumerate(blocks):
            tq = consts.tile([rows, E], BF16, tag=f"wq{bi}")
            nc.sync.dma_start(tq[:], wqa_d[rowoff[bi] : rowoff[bi] + rows, :])
            wq_t.append(tq)
            tv = consts.tile([rows, E], BF16, tag=f"wv{bi}")
            nc.sync.dma_start(tv[:], wva_d[rowoff[bi] : rowoff[bi] + rows, :])
            wv_t.append(tv)
        wos_t = []
        for bi in range(6):
            t = consts.tile([64, C], BF16, tag=f"wos{bi}")
            nc.sync.dma_start(t[:], wos_d[64 * bi : 64 * (bi + 1), :])
            wos_t.append(t)

        # ---- persistent SBUF tensors ----
        q0 = big.tile([128, NP], BF16, tag="q0")
        q1 = big.tile([128, NP], BF16, tag="q1")
        q2a = big.tile([64, NP], BF16, tag="q2a")
        q2b = big.tile([64, NP], BF16, tag="q2b")
        v_sb = big.tile([128, NTILE, E], BF16, tag="v_sb")
        k_sb = big.tile([128, NTILE, E], BF16, tag="k_sb")
        clf = big.tile([128, NP], BF16, tag="clf")
        slf = big.tile([128, NP], BF16, tag="slf")
        clh = big.tile([64, NP], BF16, tag="clh")
        slh = big.tile([64, NP], BF16, tag="slh")

        # trig + k loads (node-major k: partition = node % 128)
        nc.sync.dma_start(clf[:], cl_d[0:128, :])
        nc.sync.dma_start(clh[:], cl_d[128:NT, :])
        nc.sync.dma_start(slf[:], sl_d[0:128, :])
        nc.sync.dma_start(slh[:], sl_d[128:NT, :])
        kview = kn_d[:].rearrange("(t p) e -> p t e", p=128)
        nc.sync.dma_start(k_sb[:], kview)

        arenas = []
        for s in range(2):
            row_set = []
            for pr in range(6):
                a = aren.tile([64, 64], BF16, tag=f"arena{s}_{pr}")
                nc.vector.memset(a[:], 0.0)
                row_set.append(a)
            arenas.append(row_set)

        # ------------------------------------------------------------------
        # Stage 1: Q/V projections + rope (per chunk of W nodes)
        # ------------------------------------------------------------------
        for ch in range(NCH):
            n0 = ch * W
            xt_c = []
            for bi, rows in enumerate(blocks):
                t = xtp.tile([rows, W], BF16, tag=f"xt{bi}")
                nc.sync.dma_start(
                    t[:], xt_d[rowoff[bi] : rowoff[bi] + rows, n0 : n0 + W]
                )
                xt_c.append(t)

            # Q projection (A-ordered columns)
            qE0 = pbank("T0")
            qO0 = pbank("T1")
            qE2 = pbank("T2")
            qO2 = pbank("T3")
            for ps, c0, m in (
                (qE0, 0, 128),
                (qO0, 128, 128),
                (qE2, 256, 64),
                (qO2, 320, 64),
            ):
                for ki in range(nk):
                    nc.tensor.matmul(
                        ps[0:m, :],
                        wq_t[ki][:, c0 : c0 + m],
                        xt_c[ki][:],
                        start=(ki == 0),
                        stop=(ki == nk - 1),
                    )

            # rope: cast PSUM->SBUF bf16, then tensor ops
            qE0s = qsb.tile([128, W], BF16, tag="qE0s")
            qO0s = qsb.tile([128, W], BF16, tag="qO0s")
            qE2s = qsb.tile([64, W], BF16, tag="qE2s")
            qO2s = qsb.tile([64, W], BF16, tag="qO2s")
            nc.scalar.activation(qE0s[:], qE0[:], AF.Copy)
            nc.vector.tensor_copy(qO0s[:], qO0[:])
            nc.scalar.activation(qE2s[:], qE2[0:64, :], AF.Copy)
            nc.vector.tensor_copy(qO2s[:], qO2[0:64, :])

            cw = lambda t: t[:, n0 : n0 + W]
            ta = qsb.tile([128, W], BF16, tag="ta")
            tb = qsb.tile([128, W], BF16, tag="tb")
            nc.vector.tensor_mul(ta[:], qE0s[:], cw(clf))
            nc.vector.tensor_mul(tb[:], qO0s[:], cw(slf))
            nc.vector.tensor_sub(cw(q0), ta[:], tb[:])
            nc.vector.tensor_mul(ta[:], qE0s[:], cw(slf))
            nc.vector.tensor_mul(tb[:], qO0s[:], cw(clf))
            nc.vector.tensor_add(cw(q1), ta[:], tb[:])
            tc2 = qsb.tile([64, W], BF16, tag="tc2")
            td2 = qsb.tile([64, W], BF16, tag="td2")
            nc.gpsimd.tensor_mul(tc2[:], qE2s[:], cw(clh))
            nc.gpsimd.tensor_mul(td2[:], qO2s[:], cw(slh))
            nc.gpsimd.tensor_sub(cw(q2a), tc2[:], td2[:])
            nc.gpsimd.tensor_mul(tc2[:], qE2s[:], cw(slh))
            nc.gpsimd.tensor_mul(td2[:], qO2s[:], cw(clh))
            nc.gpsimd.tensor_add(cw(q2b), tc2[:], td2[:])

            # V per node tile
            for sub in range(W // 128):
                ti = ch * (W // 128) + sub
                f0 = sub * 128
                vps = pbank("T4")
                for ki in range(nk):
                    nc.tensor.matmul(
                        vps[:, 0:E],
                        xt_c[ki][:, f0 : f0 + 128],
                        wv_t[ki][:],
                        start=(ki == 0),
                        stop=(ki == nk - 1),
                    )
                nc.scalar.activation(v_sb[:, ti, :], vps[:, 0:E], AF.Copy)

        # ------------------------------------------------------------------
        # Stage 2+3 per graph: KV, M_b, out matmuls
        # ------------------------------------------------------------------
        oti = 0
        for j in range(GPD):
            t0 = j * TPS
            kvt = [pbank("T0"), pbank("T1"), pbank("T2")]
            for cchunk in range(3):
                for tt in range(TPS):
                    nc.tensor.matmul(
                        kvt[cchunk][:, 0:128],
                        v_sb[:, t0 + tt, 128 * cchunk : 128 * (cchunk + 1)],
                        k_sb[:, t0 + tt, 128 * cchunk : 128 * (cchunk + 1)],
                        start=(tt == 0),
                        stop=(tt == TPS - 1),
                    )

            ars = arenas[j % 2]
            for g in range(G):
                cchunk, m = divmod(g, 4)
                pr, par = divmod(g, 2)
                src = kvt[cchunk][32 * m : 32 * m + 32, 32 * m : 32 * m + 32]
                dst = ars[pr][32 * par : 32 * par + 32, :].rearrange(
                    "e (h s) -> e h s", s=16
                )[:, par::2, :]
                nc.vector.tensor_copy(dst, src.rearrange("e (h s) -> e h s", s=16))

            mb_ps = [pbank("T3"), pbank("T4")]
            for cch in range(2):  # chunk0 = E rows, chunk1 = O rows (t<128)
                colsel = slice(0, 32) if cch == 0 else slice(32, 64)
                for j2 in range(4):
                    nc.tensor.matmul(
                        mb_ps[cch][32 * j2 : 32 * j2 + 32, 0:C],
                        ars[j2][:, colsel],
                        wos_t[j2][:],
                        start=True,
                        stop=True,
                        tile_position=(0, 32 * j2),
                    )
            # chunk2: E rows (t>=128) and O rows in separate banks, base 0
            mb_ps2 = [pbank("T5"), pbank("T6")]
            for half, colsel in ((0, slice(0, 32)), (1, slice(32, 64))):
                for sub in range(2):
                    nc.tensor.matmul(
                        mb_ps2[half][32 * sub : 32 * sub + 32, 0:C],
                        ars[4 + sub][:, colsel],
                        wos_t[4 + sub][:],
                        start=True,
                        stop=True,
                        tile_position=(0, 32 * sub),
                    )

            mb0 = mbp.tile([128, C], BF16, tag="mb0")
            mb1 = mbp.tile([128, C], BF16, tag="mb1")
            mb2a = mbp.tile([64, C], BF16, tag="mb2a")
            mb2b = mbp.tile([64, C], BF16, tag="mb2b")
            nc.scalar.activation(mb0[:], mb_ps[0][:, 0:C], AF.Copy)
            nc.vector.tensor_copy(mb1[:], mb_ps[1][:, 0:C])
            nc.scalar.activation(mb2a[:], mb_ps2[0][0:64, 0:C], AF.Copy)
            nc.vector.tensor_copy(mb2b[:], mb_ps2[1][0:64, 0:C])

            slot0 = j * slot
            wins = []
            o = 0
            while o < slot:
                w = min(W, slot - o)
                wins.append((slot0 + o, w))
                o += w
            for cch in range(3):
                cc = slice(128 * cch, 128 * (cch + 1))
                for w0, w in wins:
                    ot = pbank(f"T{6 + (oti % 2)}")
                    nc.tensor.matmul(
                        ot[:, :w], mb0[:, cc], q0[:, w0 : w0 + w], start=True, stop=False
                    )
                    nc.tensor.matmul(
                        ot[:, :w], mb1[:, cc], q1[:, w0 : w0 + w], start=False, stop=False
                    )
                    nc.tensor.matmul(
                        ot[:, :w], mb2a[:, cc], q2a[:, w0 : w0 + w], start=False, stop=False
                    )
                    nc.tensor.matmul(
                        ot[:, :w], mb2b[:, cc], q2b[:, w0 : w0 + w], start=False, stop=True
                    )
                    ost = outp.tile([128, W], FP32, tag=f"ost{oti % 2}")
                    if oti % 2 == 0:
                        nc.vector.tensor_copy(ost[:, :w], ot[:, :w])
                    else:
                        nc.scalar.activation(ost[:, :w], ot[:, :w], AF.Copy)
                    nc.sync.dma_start(out_d[cch, :, w0 : w0 + w], ost[:, :w])
                    oti += 1

    nc.compile()

    _CACHE[key] = (nc, NP)
    return nc, NP


last_exec_time_ns = None


def kernel(x, pos, batch, Wq, bq, Wv, bv, Wo, bo, freqs):
    global last_exec_time_ns
    x = np.asarray(x, dtype=np.float32)
    pos = np.asarray(pos, dtype=np.float32)
    batch = np.asarray(batch).astype(np.int64)
    Wq = np.asarray(Wq, dtype=np.float32)
    bq = np.asarray(bq, dtype=np.float32)
    Wv = np.asarray(Wv, dtype=np.float32)
    bv = np.asarray(bv, dtype=np.float32)
    Wo = np.asarray(Wo, dtype=np.float32)
    bo = np.asarray(bo, dtype=np.float32)
    freqs = np.asarray(freqs, dtype=np.float32)

    counts = np.bincount(batch, minlength=NUM_GRAPHS)
    starts = np.concatenate([[0], np.cumsum(counts)])
    slot = max(640, int(math.ceil(counts.max() / 128.0)) * 128)
    has_bias = bool(np.any(bq) or np.any(bv))

    nc, NP = _build(slot, has_bias)

    WqA = Wq[:, _APERM]
    bqA = bq[_APERM]
    CA = C + 1 if has_bias else C
    bf = ml_dtypes.bfloat16

    wqa = np.zeros((CA, E), dtype=bf)
    wqa[:C] = WqA.astype(bf)
    wva = np.zeros((CA, E), dtype=bf)
    wva[:C] = Wv.astype(bf)
    if has_bias:
        wqa[C] = bqA.astype(bf)
        wva[C] = bv.astype(bf)
    wos = (Wo * (math.sqrt(2.0) / AVG)).astype(bf)

    # phase & trig on host (t = g*16+p, g-major)
    fr = freqs.reshape(NT, SD)
    phase = pos @ fr.T  # [N, 192] float32
    cphase = np.cos(phase)
    sphase = np.sin(phase)
    # k node-major, G-order columns: kE -> col g*32+p, kO -> col g*32+16+p
    s2 = 1.0 / math.sqrt(2.0)
    kfull = np.empty((len(x), E), dtype=np.float32)
    k3 = kfull.reshape(len(x), G, D)
    ph3c = cphase.reshape(len(x), G, P)
    ph3s = sphase.reshape(len(x), G, P)
    k3[:, :, 0:P] = (ph3c - ph3s) * s2
    k3[:, :, P:D] = (ph3c + ph3s) * s2

    in_maps = []
    for d in range(NCORES):
        xt = np.zeros((CA, NP), dtype=bf)
        cl = np.zeros((NT, NP), dtype=bf)
        sl = np.zeros((NT, NP), dtype=bf)
        kn = np.zeros((NP, E), dtype=bf)
        for lj in range(GPD):
            gb = d * GPD + lj
            s, e_, cnt = starts[gb], starts[gb + 1], counts[gb]
            if cnt == 0:
                continue
            o = lj * slot
            xt[:C, o : o + cnt] = x[s:e_].T.astype(bf)
            if has_bias:
                xt[C, o : o + cnt] = 1.0
            cl[:, o : o + cnt] = cphase[s:e_].T.astype(bf)
            sl[:, o : o + cnt] = sphase[s:e_].T.astype(bf)
            kn[o : o + cnt, :] = kfull[s:e_].astype(bf)
        in_maps.append(
            {
                "xt": xt,
                "cl": cl,
                "sl": sl,
                "kn": kn,
                "wqa": wqa,
                "wva": wva,
                "wos": wos,
            }
        )

    want_trace = bool(int(os.environ.get("PLATCONV_TRACE", "0")))
    if want_trace:
        want_trace = _ensure_ntff_hook()
    res = run_bass_kernel_spmd(
        nc,
        in_maps,
        core_ids=list(range(NCORES)),
        trace=want_trace,
    )
    last_exec_time_ns = res.exec_time_ns

    out = np.empty((N, C), dtype=np.float32)
    for d in range(NCORES):
        ot = np.asarray(res.results[d]["outt"], dtype=np.float32).reshape(C, NP)
        for lj in range(GPD):
            gb = d * GPD + lj
            s, e_, cnt = starts[gb], starts[gb + 1], counts[gb]
            if cnt == 0:
                continue
            o = lj * slot
            out[s:e_] = ot[:, o : o + cnt].T
    out += bo[None, :]
    return out


# revision 20
# speedup vs baseline: 1.1287x; 1.1287x over previous
"""Trainium2 Bass kernel for nn_PlatonicConv (linear-attention GNN message passing).

Math (reference):
  q = rope(x@Wq + bq, phase);  k = rope(ones, phase);  v = x@Wv + bv
  phase[n, g, p] = pos[n, :] . freqs[g, 0, p, :]
  KV_b[g] = (1/AVG) * sum_{n in graph b} k[n,g,:] (x) v[n,g,:]
  out[n]  = concat_g( q'[n,g,:] @ KV_b[g] ) @ Wo + bo

Device formulation (per core, data-parallel over graphs; 8 graphs/core):
  - host precomputes trig: cos/sin of phase (feature-major "layout L" for the
    q rope) and k = rope(ones) (node-major, 1/sqrt2 folded into Wo scale).
    (ACT's Sin LUT is only valid on [-pi, pi]; phases exceed that.)
  - Per graph b:  M_b = stack_g(KV_b[g] @ Wo[g-rows]) : [384, 384]
    out[n] = q'[n] @ M_{b(n)}  (+ bo on host).
  - q'/M_b rows use "A-order" over rope pairs t = g*16+p:
      rows   0:128 = E_t (even dims), t=0..127
      rows 128:256 = O_t (odd  dims), t=0..127
      rows 256:320 = E_t, t=128..191   (separate 64-partition tiles)
      rows 320:384 = O_t, t=128..191
  - k/v tiles are node-major; KV^T computed with dense 4-group blocks on PE;
    diagonal 32x32 blocks copied into pre-zeroed block-diagonal "arena"
    stationaries for the col-tiled M_b matmuls.

Self-contained: hardcodes shapes; shards/pads on host inside kernel().
"""

import math
import os
from contextlib import ExitStack

import ml_dtypes
import numpy as np

import concourse.bacc as bacc_mod
import concourse.bass as bass
import concourse.mybir as mybir
import concourse.tile as tile
from concourse.bass_utils import run_bass_kernel_spmd


def _ensure_ntff_hook():
    """Register the axon NTFF profile hook if the image's antenv lacks it."""
    try:
        import antenv.axon_hooks  # noqa: F401

        return True
    except ImportError:
        pass
    try:
        import sys
        import types

        import antenv
        from trn_agent_boot.trn_boot import _ntff_profile_via_ctypes

        mod = types.ModuleType("antenv.axon_hooks")
        _hook = [None]
        mod.set_axon_ntff_profile_hook = lambda h: _hook.__setitem__(0, h)
        mod.get_axon_ntff_profile_hook = lambda: _hook[0]
        sys.modules["antenv.axon_hooks"] = mod
        antenv.axon_hooks = mod
        mod.set_axon_ntff_profile_hook(
            _ntff_profile_via_ctypes("/opt/axon/libaxon_pjrt.so")
        )
        return True
    except Exception:
        return False


FP32 = mybir.dt.float32
BF16 = mybir.dt.bfloat16
AF = mybir.ActivationFunctionType

N = 32768
C = 384
E = 384
G = 12
D = 32
P = 16
SD = 3
NUM_GRAPHS = 64
NCORES = 8
GPD = NUM_GRAPHS // NCORES  # graphs per device
AVG = float(N) / NUM_GRAPHS  # 512.0
NT = 192  # rope pairs = G*P
W = 512  # streaming window


def _a_order_cols():
    """perm such that A-order column r is original q-dim perm[r]."""
    perm = np.empty(E, dtype=np.int64)
    for r in range(E):
        if r < 128:
            t, odd = r, 0
        elif r < 256:
            t, odd = r - 128, 1
        elif r < 320:
            t, odd = 128 + (r - 256), 0
        else:
            t, odd = 128 + (r - 320), 1
        perm[r] = (t // 16) * 32 + 2 * (t % 16) + odd
    return perm


_APERM = _a_order_cols()

_CACHE = {}


def _build(slot: int, has_bias: bool):
    key = (slot, has_bias)
    if key in _CACHE:
        return _CACHE[key]

    NP = GPD * slot
    NTILE = NP // 128
    TPS = slot // 128
    NCH = NP // W
    assert NP % W == 0

    nc = bacc_mod.Bacc()

    CA = C + 1 if has_bias else C
    blocks = [128, 128, 128] + ([1] if has_bias else [])
    nk = len(blocks)
    rowoff = [0, 128, 256, 384]

    xt_d = nc.declare_dram_parameter("xt", [CA, NP], BF16, isOutput=False)
    cl_d = nc.declare_dram_parameter("cl", [NT, NP], BF16, isOutput=False)
    sl_d = nc.declare_dram_parameter("sl", [NT, NP], BF16, isOutput=False)
    kn_d = nc.declare_dram_parameter("kn", [NP, E], BF16, isOutput=False)
    wqa_d = nc.declare_dram_parameter("wqa", [CA, E], BF16, isOutput=False)
    wva_d = nc.declare_dram_parameter("wva", [CA, E], BF16, isOutput=False)
    wos_d = nc.declare_dram_parameter("wos", [E, C], BF16, isOutput=False)
    out_d = nc.declare_dram_parameter("outt", [3, 128, NP], FP32, isOutput=True)

    with ExitStack() as ctx:
        tc = ctx.enter_context(tile.TileContext(nc))

        consts = ctx.enter_context(tc.tile_pool(name="consts", bufs=1))
        xtp = ctx.enter_context(tc.tile_pool(name="xtp", bufs=3))
        qsb = ctx.enter_context(tc.tile_pool(name="qsb", bufs=2))
        big = ctx.enter_context(tc.tile_pool(name="big", bufs=1))
        aren = ctx.enter_context(tc.tile_pool(name="aren", bufs=1))
        mbp = ctx.enter_context(tc.tile_pool(name="mbp", bufs=2))
        outp = ctx.enter_context(tc.tile_pool(name="outp", bufs=4))
        psum = ctx.enter_context(tc.tile_pool(name="psum", bufs=1, space="PSUM"))

        def pbank(tag):
            """Allocate one full PSUM bank ([128, 512] f32)."""
            return psum.tile([128, W], FP32, tag=tag, name=tag)

        # ---- constants (weights) ----
        wq_t, wv_t = [], []
        for bi, rows in enumerate(blocks):
            tq = consts.tile([rows, E], BF16, tag=f"wq{bi}")
            nc.sync.dma_start(tq[:], wqa_d[rowoff[bi] : rowoff[bi] + rows, :])
            wq_t.append(tq)
            tv = consts.tile([rows, E], BF16, tag=f"wv{bi}")
            nc.sync.dma_start(tv[:], wva_d[rowoff[bi] : rowoff[bi] + rows, :])
            wv_t.append(tv)
        wos_t = []
        for bi in range(6):
            t = consts.tile([64, C], BF16, tag=f"wos{bi}")
            nc.sync.dma_start(t[:], wos_d[64 * bi : 64 * (bi + 1), :])
            wos_t.append(t)

        # ---- persistent SBUF tensors ----
        q0 = big.tile([128, NP], BF16, tag="q0")
        q1 = big.tile([128, NP], BF16, tag="q1")
        q2a = big.tile([64, NP], BF16, tag="q2a")
        q2b = big.tile([64, NP], BF16, tag="q2b")
        v_sb = big.tile([128, NTILE, E], BF16, tag="v_sb")
        k_sb = big.tile([128, NTILE, E], BF16, tag="k_sb")
        clf = big.tile([128, NP], BF16, tag="clf")
        slf = big.tile([128, NP], BF16, tag="slf")
        clh = big.tile([64, NP], BF16, tag="clh")
        slh = big.tile([64, NP], BF16, tag="slh")

        # trig + k loads (node-major k: partition = node % 128)
        nc.sync.dma_start(clf[:], cl_d[0:128, :])
        nc.sync.dma_start(clh[:], cl_d[128:NT, :])
        nc.sync.dma_start(slf[:], sl_d[0:128, :])
        nc.sync.dma_start(slh[:], sl_d[128:NT, :])
        kview = kn_d[:].rearrange("(t p) e -> p t e", p=128)
        nc.sync.dma_start(k_sb[:], kview)

        arenas = []
        for s in range(2):
            row_set = []
            for pr in range(6):
                a = aren.tile([64, 64], BF16, tag=f"arena{s}_{pr}")
                nc.vector.memset(a[:], 0.0)
                row_set.append(a)
            arenas.append(row_set)

        # ------------------------------------------------------------------
        # Stage 1: Q/V projections + rope (per chunk of W nodes)
        # ------------------------------------------------------------------
        for ch in range(NCH):
            n0 = ch * W
            xt_c = []
            for bi, rows in enumerate(blocks):
                t = xtp.tile([rows, W], BF16, tag=f"xt{bi}")
                nc.sync.dma_start(
                    t[:], xt_d[rowoff[bi] : rowoff[bi] + rows, n0 : n0 + W]
                )
                xt_c.append(t)

            # Q projection (A-ordered columns)
            qE0 = pbank("T0")
            qO0 = pbank("T1")
            qE2 = pbank("T2")
            qO2 = pbank("T3")
            for ps, c0, m in (
                (qE0, 0, 128),
                (qO0, 128, 128),
                (qE2, 256, 64),
                (qO2, 320, 64),
            ):
                for ki in range(nk):
                    nc.tensor.matmul(
                        ps[0:m, :],
                        wq_t[ki][:, c0 : c0 + m],
                        xt_c[ki][:],
                        start=(ki == 0),
                        stop=(ki == nk - 1),
                    )

            # rope: cast PSUM->SBUF bf16, then tensor ops
            qE0s = qsb.tile([128, W], BF16, tag="qE0s")
            qO0s = qsb.tile([128, W], BF16, tag="qO0s")
            qE2s = qsb.tile([64, W], BF16, tag="qE2s")
            qO2s = qsb.tile([64, W], BF16, tag="qO2s")
            nc.scalar.activation(qE0s[:], qE0[:], AF.Copy)
            nc.vector.tensor_copy(qO0s[:], qO0[:])
            nc.scalar.activation(qE2s[:], qE2[0:64, :], AF.Copy)
            nc.vector.tensor_copy(qO2s[:], qO2[0:64, :])

            cw = lambda t: t[:, n0 : n0 + W]
            ta = qsb.tile([128, W], BF16, tag="ta")
            tb = qsb.tile([128, W], BF16, tag="tb")
            nc.vector.tensor_mul(ta[:], qE0s[:], cw(clf))
            nc.vector.tensor_mul(tb[:], qO0s[:], cw(slf))
            nc.vector.tensor_sub(cw(q0), ta[:], tb[:])
            nc.vector.tensor_mul(ta[:], qE0s[:], cw(slf))
            nc.vector.tensor_mul(tb[:], qO0s[:], cw(clf))
            nc.vector.tensor_add(cw(q1), ta[:], tb[:])
            tc2 = qsb.tile([64, W], BF16, tag="tc2")
            td2 = qsb.tile([64, W], BF16, tag="td2")
            nc.gpsimd.tensor_mul(tc2[:], qE2s[:], cw(clh))
            nc.gpsimd.tensor_mul(td2[:], qO2s[:], cw(slh))
            nc.gpsimd.tensor_sub(cw(q2a), tc2[:], td2[:])
            nc.gpsimd.tensor_mul(tc2[:], qE2s[:], cw(slh))
            nc.gpsimd.tensor_mul(td2[:], qO2s[:], cw(clh))
            nc.gpsimd.tensor_add(cw(q2b), tc2[:], td2[:])

            # V per node tile
            for sub in range(W // 128):
                ti = ch * (W // 128) + sub
                f0 = sub * 128
                vps = pbank("T4")
                for ki in range(nk):
                    nc.tensor.matmul(
                        vps[:, 0:E],
                        xt_c[ki][:, f0 : f0 + 128],
                        wv_t[ki][:],
                        start=(ki == 0),
                        stop=(ki == nk - 1),
                    )
                nc.scalar.activation(v_sb[:, ti, :], vps[:, 0:E], AF.Copy)

        # ------------------------------------------------------------------
        # Stage 2+3 per graph: KV, M_b, out matmuls
        # ------------------------------------------------------------------
        oti = 0
        for j in range(GPD):
            t0 = j * TPS
            kvt = [pbank("T0"), pbank("T1"), pbank("T2")]
            for cchunk in range(3):
                for tt in range(TPS):
                    nc.tensor.matmul(
                        kvt[cchunk][:, 0:128],
                        v_sb[:, t0 + tt, 128 * cchunk : 128 * (cchunk + 1)],
                        k_sb[:, t0 + tt, 128 * cchunk : 128 * (cchunk + 1)],
                        start=(tt == 0),
                        stop=(tt == TPS - 1),
                    )

            ars = arenas[j % 2]
            for g in range(G):
                cchunk, m = divmod(g, 4)
                pr, par = divmod(g, 2)
                src = kvt[cchunk][32 * m : 32 * m + 32, 32 * m : 32 * m + 32]
                dst = ars[pr][32 * par : 32 * par + 32, :].rearrange(
                    "e (h s) -> e h s", s=16
                )[:, par::2, :]
                nc.vector.tensor_copy(dst, src.rearrange("e (h s) -> e h s", s=16))

            mb_ps = [pbank("T3"), pbank("T4")]
            for cch in range(2):  # chunk0 = E rows, chunk1 = O rows (t<128)
                colsel = slice(0, 32) if cch == 0 else slice(32, 64)
                for j2 in range(4):
                    nc.tensor.matmul(
                        mb_ps[cch][32 * j2 : 32 * j2 + 32, 0:C],
                        ars[j2][:, colsel],
                        wos_t[j2][:],
                        start=True,
                        stop=True,
                        tile_position=(0, 32 * j2),
                    )
            # chunk2: E rows (t>=128) and O rows in separate banks, base 0
            mb_ps2 = [pbank("T5"), pbank("T6")]
            for half, colsel in ((0, slice(0, 32)), (1, slice(32, 64))):
                for sub in range(2):
                    nc.tensor.matmul(
                        mb_ps2[half][32 * sub : 32 * sub + 32, 0:C],
                        ars[4 + sub][:, colsel],
                        wos_t[4 + sub][:],
                        start=True,
                        stop=True,
                        tile_position=(0, 32 * sub),
                    )

            mb0 = mbp.tile([128, C], BF16, tag="mb0")
            mb1 = mbp.tile([128, C], BF16, tag="mb1")
            mb2a = mbp.tile([64, C], BF16, tag="mb2a")
            mb2b = mbp.tile([64, C], BF16, tag="mb2b")
            nc.scalar.activation(mb0[:], mb_ps[0][:, 0:C], AF.Copy)
            nc.vector.tensor_copy(mb1[:], mb_ps[1][:, 0:C])
            nc.scalar.activation(mb2a[:], mb_ps2[0][0:64, 0:C], AF.Copy)
            nc.vector.tensor_copy(mb2b[:], mb_ps2[1][0:64, 0:C])

            slot0 = j * slot
            wins = []
            o = 0
            while o < slot:
                w = min(W, slot - o)
                wins.append((slot0 + o, w))
                o += w
            for cch in range(3):
                cc = slice(128 * cch, 128 * (cch + 1))
                for w0, w in wins:
                    ot = pbank(f"T{6 + (oti % 2)}")
                    nc.tensor.matmul(
                        ot[:, :w], mb0[:, cc], q0[:, w0 : w0 + w], start=True, stop=False
                    )
                    nc.tensor.matmul(
                        ot[:, :w], mb1[:, cc], q1[:, w0 : w0 + w], start=False, stop=False
                    )
                    nc.tensor.matmul(
                        ot[:, :w], mb2a[:, cc], q2a[:, w0 : w0 + w], start=False, stop=False
                    )
                    nc.tensor.matmul(
                        ot[:, :w], mb2b[:, cc], q2b[:, w0 : w0 + w], start=False, stop=True
                    )
                    ost = outp.tile([128, W], FP32, tag=f"ost{oti % 2}")
                    if oti % 2 == 0:
                        nc.vector.tensor_copy(ost[:, :w], ot[:, :w])
                    else:
                        nc.scalar.activation(ost[:, :w], ot[:, :w], AF.Copy)
                    nc.sync.dma_start(out_d[cch, :, w0 : w0 + w], ost[:, :w])
                    oti += 1

    nc.compile()

    _CACHE[key] = (nc, NP)
    return nc, NP


last_exec_time_ns = None
last_results = None


def kernel(x, pos, batch, Wq, bq, Wv, bv, Wo, bo, freqs):
    global last_exec_time_ns
    x = np.asarray(x, dtype=np.float32)
    pos = np.asarray(pos, dtype=np.float32)
    batch = np.asarray(batch).astype(np.int64)
    Wq = np.asarray(Wq, dtype=np.float32)
    bq = np.asarray(bq, dtype=np.float32)
    Wv = np.asarray(Wv, dtype=np.float32)
    bv = np.asarray(bv, dtype=np.float32)
    Wo = np.asarray(Wo, dtype=np.float32)
    bo = np.asarray(bo, dtype=np.float32)
    freqs = np.asarray(freqs, dtype=np.float32)

    counts = np.bincount(batch, minlength=NUM_GRAPHS)
    starts = np.concatenate([[0], np.cumsum(counts)])
    slot = max(640, int(math.ceil(counts.max() / 128.0)) * 128)
    has_bias = bool(np.any(bq) or np.any(bv))

    nc, NP = _build(slot, has_bias)

    WqA = Wq[:, _APERM]
    bqA = bq[_APERM]
    CA = C + 1 if has_bias else C
    bf = ml_dtypes.bfloat16

    wqa = np.zeros((CA, E), dtype=bf)
    wqa[:C] = WqA.astype(bf)
    wva = np.zeros((CA, E), dtype=bf)
    wva[:C] = Wv.astype(bf)
    if has_bias:
        wqa[C] = bqA.astype(bf)
        wva[C] = bv.astype(bf)
    wos = (Wo * (math.sqrt(2.0) / AVG)).astype(bf)

    # phase & trig on host (t = g*16+p, g-major)
    fr = freqs.reshape(NT, SD)
    phase = pos @ fr.T  # [N, 192] float32
    cphase = np.cos(phase)
    sphase = np.sin(phase)
    # k node-major, G-order columns: kE -> col g*32+p, kO -> col g*32+16+p
    s2 = 1.0 / math.sqrt(2.0)
    kfull = np.empty((len(x), E), dtype=np.float32)
    k3 = kfull.reshape(len(x), G, D)
    ph3c = cphase.reshape(len(x), G, P)
    ph3s = sphase.reshape(len(x), G, P)
    k3[:, :, 0:P] = (ph3c - ph3s) * s2
    k3[:, :, P:D] = (ph3c + ph3s) * s2

    in_maps = []
    for d in range(NCORES):
        xt = np.zeros((CA, NP), dtype=bf)
        cl = np.zeros((NT, NP), dtype=bf)
        sl = np.zeros((NT, NP), dtype=bf)
        kn = np.zeros((NP, E), dtype=bf)
        for lj in range(GPD):
            gb = d * GPD + lj
            s, e_, cnt = starts[gb], starts[gb + 1], counts[gb]
            if cnt == 0:
                continue
            o = lj * slot
            xt[:C, o : o + cnt] = x[s:e_].T.astype(bf)
            if has_bias:
                xt[C, o : o + cnt] = 1.0
            cl[:, o : o + cnt] = cphase[s:e_].T.astype(bf)
            sl[:, o : o + cnt] = sphase[s:e_].T.astype(bf)
            kn[o : o + cnt, :] = kfull[s:e_].astype(bf)
        in_maps.append(
            {
                "xt": xt,
                "cl": cl,
                "sl": sl,
                "kn": kn,
                "wqa": wqa,
                "wva": wva,
                "wos": wos,
            }
        )

    want_trace = bool(int(os.environ.get("PLATCONV_TRACE", "0")))
    if want_trace:
        want_trace = _ensure_ntff_hook()
    res = run_bass_kernel_spmd(
        nc,
        in_maps,
        core_ids=list(range(NCORES)),
        trace=want_trace,
    )
    last_exec_time_ns = res.exec_time_ns
    global last_results
    last_results = res

    out = np.zeros((N, C), dtype=np.float32)
    for d in range(NCORES):
        ot = np.asarray(res.results[d]["outt"], dtype=np.float32).reshape(C, NP)
        for lj in range(GPD):
            gb = d * GPD + lj
            s, e_, cnt = starts[gb], starts[gb + 1], counts[gb]
            if cnt == 0:
                continue
            o = lj * slot
            out[s:e_] = ot[:, o : o + cnt].T
    out += bo[None, :]
    return out
